# revision 1
# baseline (speedup 1.0000x reference)
"""Trainium2 Bass kernel for nn_CrossAttentionBlock (B=4, C=512, H=W=64).

Decomposition across 8 NeuronCores: core = (batch b, query-half h).
Each core:
  stage 1: theta/phi = conv1x1(x1) packed as one [128-out] projection (PE)
  stage 2: g^T = conv1x1(x0) in [m, 64] layout + ones column (PE)
  main:    fT[m, n] = theta^T phi (PE, keys on partitions), p = exp(fT) (ACT),
           yT_ext = [g, 1]^T p accumulated over key chunks (PE) -> softmax
           numerator rows 0..63 and denominator row 64 in one accumulation.
  gather:  transpose yT -> y rows, normalize by denominator, + g_b,
           pair-wise AllGather assembles the full y for the batch.
  phase 2: W_y = W [view of y] consumed only as per-channel bn stats (AdaIN
           needs only mean/var of W_y); x0 instance stats; final out =
           r * x0 + t with per-channel scalars.

SPMD uniformity: the key/spatial axis m and the channel axis c are dummy
(contraction/stat) indices, so each core receives inputs permuted so that
"its" queries and "its" output channels come first; the host un-permutes
the output columns.
"""
import numpy as np
from contextlib import ExitStack

import concourse.bass as bass
import concourse.tile as tile
from concourse import mybir
from concourse.bass_utils import run_bass_kernel_spmd

FP32 = mybir.dt.float32
ALU = mybir.AluOpType
ACTF = mybir.ActivationFunctionType

B, C, H, W = 4, 512, 64, 64
N = H * W          # 4096 tokens
C8 = C // 8        # 64 inner channels
NH = N // 2        # 2048 queries per core
OC = C // 2        # 256 output channels per core
EPS = 1e-5

REPLICA_PAIRS = [[0, 1], [2, 3], [4, 5], [6, 7]]


def _split_excess_waits(nc, max_waits=1, drain_max=1):
    """walrus here rejects instructions carrying more than ~2 sync waits; move
    extras to preceding NoOps on the same engine (semantics preserved: waits
    run before the instruction, engine streams are sequential)."""
    for blk in nc.main_func.blocks:
        insts = blk.instructions
        k = 0
        while k < len(insts):
            inst = insts[k]
            si = inst.sync_info
            cap = drain_max if inst.opcode == "Drain" else max_waits
            if si is not None and si.on_wait and len(si.on_wait) > cap:
                waits = list(si.on_wait)
                keep = waits[-cap:]
                extra = waits[:-cap]
                pos = k
                for j in range(0, len(extra), cap):
                    nop = mybir.InstNoOp(name=f"{inst.name}-wsplit{j}", ins=[], outs=[])
                    nop.engine = inst.engine
                    nop.sync_info = mybir.SyncInfo(
                        on_wait=extra[j : j + cap], on_update=[]
                    )
                    insts.insert(pos, nop)
                    pos += 1
                    k += 1
                inst.sync_info = mybir.SyncInfo(on_wait=keep, on_update=list(si.on_update))
            k += 1


def build_nc():
    nc = bass.Bass()

    x0 = nc.dram_tensor("x0", [C, N], FP32, kind="ExternalInput")
    x1 = nc.dram_tensor("x1", [C, N], FP32, kind="ExternalInput")
    tp_wT = nc.dram_tensor("tp_wT", [C, 128], FP32, kind="ExternalInput")
    tp_b = nc.dram_tensor("tp_b", [128, 1], FP32, kind="ExternalInput")
    g_wT = nc.dram_tensor("g_wT", [C, C8], FP32, kind="ExternalInput")
    g_b_bc = nc.dram_tensor("g_b_bc", [128, C8], FP32, kind="ExternalInput")
    W_wTh = nc.dram_tensor("W_wTh", [C8, OC], FP32, kind="ExternalInput")
    W_bh = nc.dram_tensor("W_bh", [128, 2], FP32, kind="ExternalInput")
    ident = nc.dram_tensor("ident", [C8 + 1, C8 + 1], FP32, kind="ExternalInput")
    out = nc.dram_tensor("out", [OC, N], FP32, kind="ExternalOutput")

    y_bounce = nc.dram_tensor("y_bounce", [NH, C8], FP32)
    y_full = nc.dram_tensor("y_full", [N, C8], FP32)

    with tile.TileContext(nc) as tc, ExitStack() as ctx:
        wpool = ctx.enter_context(tc.tile_pool(name="weights", bufs=1))
        big = ctx.enter_context(tc.tile_pool(name="big", bufs=1))

        # ---- weights to SBUF ----
        tp_w_sb = wpool.tile([128, 4, 128], FP32)
        g_w_sb = wpool.tile([128, 4, C8], FP32)
        for c in range(4):
            nc.sync.dma_start(out=tp_w_sb[:, c, :], in_=tp_wT[c * 128:(c + 1) * 128, :])
            nc.sync.dma_start(out=g_w_sb[:, c, :], in_=g_wT[c * 128:(c + 1) * 128, :])
        tp_b_sb = wpool.tile([128, 1], FP32)
        nc.sync.dma_start(out=tp_b_sb[:], in_=tp_b[:])
        g_b_sb = wpool.tile([128, C8], FP32)
        nc.sync.dma_start(out=g_b_sb[:], in_=g_b_bc[:])
        W_w_sb = wpool.tile([C8, OC], FP32)
        nc.sync.dma_start(out=W_w_sb[:], in_=W_wTh[:])
        W_b_sb = wpool.tile([128, 2], FP32)
        nc.sync.dma_start(out=W_b_sb[:], in_=W_bh[:])
        id_sb = wpool.tile([C8 + 1, C8 + 1], FP32)
        nc.sync.dma_start(out=id_sb[:], in_=ident[:])

        # ---- persistent big tensors ----
        x0_sb = big.tile([128, 4, N], FP32)      # c-chunk on middle index
        theta_sb = big.tile([C8, N], FP32)       # keys, [64, 4096]
        phi_sb = big.tile([C8, NH], FP32)        # queries (own half), [64, 2048]
        g_extT = big.tile([128, 32, C8 + 1], FP32)  # [m-chunk, 65] per chunk
        yT_sb = big.tile([C8 + 1, NH], FP32)
        yv_sb = big.tile([C8, N], FP32)          # gathered y viewed [64, 4096]

        nc.gpsimd.memset(g_extT[:, :, C8:C8 + 1], 1.0)

        # ---- stage 1: x1 -> theta/phi ----
        with tc.tile_pool(name="x1blk", bufs=8) as x1pool, \
             tc.tile_pool(name="ps_tp", bufs=2, space="PSUM") as ps_tp:
            for blk in range(8):
                cols = slice(blk * 512, (blk + 1) * 512)
                xt = []
                for c in range(4):
                    t = x1pool.tile([128, 512], FP32)
                    nc.sync.dma_start(out=t[:], in_=x1[c * 128:(c + 1) * 128, cols])
                    xt.append(t)
                ptp = ps_tp.tile([128, 512], FP32)
                for c in range(4):
                    nc.tensor.matmul(ptp[:], tp_w_sb[:, c, :], xt[c][:],
                                     start=(c == 0), stop=(c == 3))
                nc.vector.tensor_scalar_add(theta_sb[:, cols], ptp[0:C8, :],
                                            tp_b_sb[0:C8, :])
                if blk < 4:
                    nc.vector.tensor_scalar_add(phi_sb[:, cols], ptp[C8:128, :],
                                                tp_b_sb[C8:128, :])

        # ---- stage 2: x0 -> g^T (transposed layout) ----
        with tc.tile_pool(name="ps_g", bufs=2, space="PSUM") as ps_g:
            for blk in range(8):
                cols = slice(blk * 512, (blk + 1) * 512)
                for c in range(4):
                    nc.sync.dma_start(out=x0_sb[:, c, cols],
                                      in_=x0[c * 128:(c + 1) * 128, cols])
                for mi in range(4 * blk, 4 * blk + 4):
                    pg = ps_g.tile([128, C8], FP32)
                    for c in range(4):
                        nc.tensor.matmul(pg[:],
                                         x0_sb[:, c, mi * 128:(mi + 1) * 128],
                                         g_w_sb[:, c, :],
                                         start=(c == 0), stop=(c == 3))
                    nc.vector.tensor_copy(g_extT[:, mi, 0:C8], pg[:])

        # ---- x0 instance stats (own channels = chunks 0, 1) ----
        stat = ctx.enter_context(tc.tile_pool(name="stats", bufs=1))
        x_aggs = []
        for oc in range(2):
            xst = stat.tile([128, 8, 6], FP32)
            for mb in range(8):
                nc.vector.bn_stats(xst[:, mb, :],
                                   x0_sb[:, oc, mb * 512:(mb + 1) * 512])
            xagg = stat.tile([128, 2], FP32)
            nc.vector.bn_aggr(xagg[:], xst[:])
            x_aggs.append(xagg)

        # ---- main attention loop ----
        with tc.tile_pool(name="ps_f", bufs=2, space="PSUM") as ps_f, \
             tc.tile_pool(name="ps_y", bufs=1, space="PSUM") as ps_y, \
             tc.tile_pool(name="pT", bufs=3) as ppool:
            for q in range(2):
                qc = slice(q * 1024, (q + 1) * 1024)
                py = ps_y.tile([C8 + 1, 1024], FP32)
                for mi in range(32):
                    ft = ps_f.tile([128, 1024], FP32)
                    for s in range(2):
                        nc.tensor.matmul(
                            ft[:, s * 512:(s + 1) * 512],
                            theta_sb[:, mi * 128:(mi + 1) * 128],
                            phi_sb[:, q * 1024 + s * 512: q * 1024 + (s + 1) * 512],
                            start=True, stop=True)
                    pt = ppool.tile([128, 1024], FP32)
                    nc.scalar.activation(pt[:], ft[:], ACTF.Exp)
                    for s in range(2):
                        nc.tensor.matmul(
                            py[:, s * 512:(s + 1) * 512],
                            g_extT[:, mi, :],
                            pt[:, s * 512:(s + 1) * 512],
                            start=(mi == 0), stop=(mi == 31))
                nc.vector.tensor_copy(yT_sb[:, qc], py[:])

        # ---- transpose, normalize, exchange ----
        with tc.tile_pool(name="ps_t", bufs=2, space="PSUM") as ps_t, \
             tc.tile_pool(name="ystage", bufs=3) as ystage:
            for j in range(16):
                ptile = ps_t.tile([128, C8 + 1], FP32)
                nc.tensor.transpose(ptile[:], yT_sb[:, j * 128:(j + 1) * 128], id_sb[:])
                rec = ystage.tile([128, 1], FP32, tag="rec")
                nc.vector.reciprocal(rec[:], ptile[:, C8:C8 + 1])
                yst = ystage.tile([128, C8], FP32, tag="yst")
                nc.vector.tensor_scalar_mul(yst[:], ptile[:, 0:C8], rec[:])
                nc.vector.tensor_add(yst[:], yst[:], g_b_sb[:])
                nc.sync.dma_start(out=y_bounce[j * 128:(j + 1) * 128, :], in_=yst[:])

        nc.gpsimd.collective_compute(
            "AllGather", ALU.bypass,
            replica_groups=REPLICA_PAIRS,
            ins=[y_bounce[:]],
            outs=[y_full[:]],
        )
        nc.sync.dma_start(out=yv_sb[:],
                          in_=y_full[:].rearrange("(a b) w -> a (b w)", a=C8))

        # ---- phase 2: W_y stats + per-channel affine + output ----
        with tc.tile_pool(name="ps_W", bufs=2, space="PSUM") as ps_W, \
             tc.tile_pool(name="sc", bufs=1) as sc, \
             tc.tile_pool(name="outp", bufs=2) as outp:
            for oc in range(2):
                wst = sc.tile([128, 8, 6], FP32, tag=f"wst{oc}")
                for mb in range(8):
                    pw = ps_W.tile([128, 512], FP32)
                    nc.tensor.matmul(pw[:], W_w_sb[:, oc * 128:(oc + 1) * 128],
                                     yv_sb[:, mb * 512:(mb + 1) * 512],
                                     start=True, stop=True)
                    nc.vector.bn_stats(wst[:, mb, :], pw[:])
                wagg = sc.tile([128, 2], FP32, tag=f"wagg{oc}")
                nc.vector.bn_aggr(wagg[:], wst[:])

                # r = sqrt((var_s + eps) / (var_c + eps)); t = mu_s - r*mu_c
                vc = sc.tile([128, 1], FP32, tag=f"vc{oc}")
                nc.vector.tensor_scalar_add(vc[:], x_aggs[oc][:, 1:2], EPS)
                rc = sc.tile([128, 1], FP32, tag=f"rc{oc}")
                nc.vector.reciprocal(rc[:], vc[:])
                vs = sc.tile([128, 1], FP32, tag=f"vs{oc}")
                nc.vector.tensor_scalar_add(vs[:], wagg[:, 1:2], EPS)
                ratio = sc.tile([128, 1], FP32, tag=f"ratio{oc}")
                nc.vector.tensor_mul(ratio[:], vs[:], rc[:])
                rr = sc.tile([128, 1], FP32, tag=f"rr{oc}")
                nc.scalar.sqrt(rr[:], ratio[:])
                mus = sc.tile([128, 1], FP32, tag=f"mus{oc}")
                nc.vector.tensor_add(mus[:], wagg[:, 0:1], W_b_sb[:, oc:oc + 1])
                rmc = sc.tile([128, 1], FP32, tag=f"rmc{oc}")
                nc.vector.tensor_mul(rmc[:], rr[:], x_aggs[oc][:, 0:1])
                tt = sc.tile([128, 1], FP32, tag=f"tt{oc}")
                nc.vector.tensor_sub(tt[:], mus[:], rmc[:])

                for mb in range(4):
                    cols = slice(mb * 1024, (mb + 1) * 1024)
                    ot = outp.tile([128, 1024], FP32)
                    nc.vector.tensor_scalar(ot[:], x0_sb[:, oc, cols], rr[:], tt[:],
                                            ALU.mult, ALU.add)
                    nc.sync.dma_start(out=out[oc * 128:(oc + 1) * 128, cols], in_=ot[:])

    _split_excess_waits(nc)
    return nc


_NC_CACHE = None


def _get_nc():
    global _NC_CACHE
    if _NC_CACHE is None:
        _NC_CACHE = build_nc()
    return _NC_CACHE


def _core_inputs(x0f, x1f, tp_wT, tp_b, g_wT, g_b, W_wT, W_b, ident, core):
    b, half = core // 2, core % 2
    x0b, x1b = x0f[b], x1f[b]
    if half == 0:
        x0p = x0b
        x1p = x1b
        g_wp = g_wT
    else:
        # queries-first column permutation; own-channels-first row permutation
        x1p = np.concatenate([x1b[:, NH:], x1b[:, :NH]], axis=1)
        x0r = np.concatenate([x0b[OC:], x0b[:OC]], axis=0)
        x0p = np.concatenate([x0r[:, NH:], x0r[:, :NH]], axis=1)
        g_wp = np.concatenate([g_wT[OC:], g_wT[:OC]], axis=0)
    return {
        "x0": np.ascontiguousarray(x0p),
        "x1": np.ascontiguousarray(x1p),
        "tp_wT": tp_wT,
        "tp_b": tp_b,
        "g_wT": np.ascontiguousarray(g_wp),
        "g_b_bc": np.ascontiguousarray(np.broadcast_to(g_b, (128, C8))),
        "W_wTh": np.ascontiguousarray(W_wT[:, half * OC:(half + 1) * OC]),
        "W_bh": np.ascontiguousarray(
            W_b[half * OC:(half + 1) * OC].reshape(2, 128).T),
        "ident": ident,
    }


def kernel(x0, x1, g_w, g_b, theta_w, theta_b, phi_w, phi_b, W_w, W_b):
    x0 = np.asarray(x0, dtype=np.float32)
    x1 = np.asarray(x1, dtype=np.float32)
    x0f = x0.reshape(B, C, N)
    x1f = x1.reshape(B, C, N)
    tp_wT = np.ascontiguousarray(
        np.concatenate([theta_w, phi_w], axis=0).T.astype(np.float32))
    tp_b = np.ascontiguousarray(
        np.concatenate([theta_b, phi_b]).astype(np.float32)[:, None])
    g_wT = np.ascontiguousarray(np.asarray(g_w, np.float32).T)
    W_wT = np.ascontiguousarray(np.asarray(W_w, np.float32).T)
    ident = np.eye(C8 + 1, dtype=np.float32)
    g_b = np.asarray(g_b, np.float32)
    W_b = np.asarray(W_b, np.float32)

    in_maps = [
        _core_inputs(x0f, x1f, tp_wT, tp_b, g_wT, g_b, W_wT, W_b, ident, core)
        for core in range(8)
    ]
    nc = _get_nc()
    res = run_bass_kernel_spmd(nc, in_maps, core_ids=list(range(8)))

    out = np.empty((B, C, N), dtype=np.float32)
    for core in range(8):
        b, half = core // 2, core % 2
        o = res.results[core]["out"]
        if half == 1:
            o = np.concatenate([o[:, NH:], o[:, :NH]], axis=1)
        out[b, half * OC:(half + 1) * OC] = o
    return out.reshape(B, C, H, W)



# revision 6
# speedup vs baseline: 2.5300x; 2.5300x over previous
"""Trainium2 Bass kernel for nn_CrossAttentionBlock (B=4, C=512, H=W=64).

Decomposition across 8 NeuronCores: core = (batch b, query-half h).
v2: all matmuls in bf16 (1 cyc/row on PE vs 4 for fp32), bf16 input DMA,
conv stages interleaved into the attention loop so the in-order PE stream
never stalls on input DMA, per-query-half AllGather (bf16) so the first
exchange hides under the second half's compute.

Each core:
  interleaved: theta/phi = conv1x1(x1) (PE, bf16), gT = conv1x1(x0) rows
  main:    fT[m, n] = theta^T phi (PE), p = exp(fT) (ACT, bf16 out),
           yT_ext = [g, 1]^T p accumulated over key chunks (PE) -> softmax
           numerator rows 0..63 and denominator row 64 in one accumulation.
  per q-half: transpose yT -> y rows, normalize, + g_b, AllGather (pairwise)
  phase 2: W_y = W [view of y] consumed only as per-channel bn stats (AdaIN
           needs only mean/var of W_y); x0 instance stats; final out =
           r * x0 + t with per-channel scalars, bf16 out.

SPMD uniformity: the key/spatial axis m and the channel axis c are dummy
(contraction/stat) indices, so each core receives inputs permuted so that
"its" queries and "its" output channels come first; the host un-permutes
the output columns. W_w rows are permuted so the two AllGather chunks land
in contiguous yv row blocks.
"""
import numpy as np
import ml_dtypes
from contextlib import ExitStack

import concourse.bass as bass
import concourse.tile as tile
from concourse import mybir
from concourse.bass_utils import run_bass_kernel_spmd

FP32 = mybir.dt.float32
BF16 = mybir.dt.bfloat16
ALU = mybir.AluOpType
ACTF = mybir.ActivationFunctionType

B, C, H, W = 4, 512, 64, 64
N = H * W          # 4096 tokens
C8 = C // 8        # 64 inner channels
NH = N // 2        # 2048 queries per core
OC = C // 2        # 256 output channels per core
EPS = 1e-5

REPLICA_PAIRS = [[0, 1], [2, 3], [4, 5], [6, 7]]

# yv row blocks delivered by the two AllGathers (see _core_inputs W_p perm):
# gather q=0 -> view rows [0:16] u [32:48]; q=1 -> [16:32] u [48:64].
W_ROW_PERM = np.concatenate([
    np.arange(0, 16), np.arange(32, 48),
    np.arange(16, 32), np.arange(48, 64),
])


def _split_excess_waits(nc, max_waits=1, drain_max=1):
    """walrus here rejects instructions carrying more than ~2 sync waits; move
    extras to preceding NoOps on the same engine (semantics preserved: waits
    run before the instruction, engine streams are sequential)."""
    for blk in nc.main_func.blocks:
        insts = blk.instructions
        k = 0
        while k < len(insts):
            inst = insts[k]
            si = inst.sync_info
            cap = drain_max if inst.opcode == "Drain" else max_waits
            if si is not None and si.on_wait and len(si.on_wait) > cap:
                waits = list(si.on_wait)
                keep = waits[-cap:]
                extra = waits[:-cap]
                pos = k
                for j in range(0, len(extra), cap):
                    nop = mybir.InstNoOp(name=f"{inst.name}-wsplit{j}", ins=[], outs=[])
                    nop.engine = inst.engine
                    nop.sync_info = mybir.SyncInfo(
                        on_wait=extra[j : j + cap], on_update=[]
                    )
                    insts.insert(pos, nop)
                    pos += 1
                    k += 1
                inst.sync_info = mybir.SyncInfo(on_wait=keep, on_update=list(si.on_update))
            k += 1


def build_nc():
    nc = bass.Bass()

    x0 = nc.dram_tensor("x0", [C, N], BF16, kind="ExternalInput")
    x1 = nc.dram_tensor("x1", [C, N], BF16, kind="ExternalInput")
    tp_wT = nc.dram_tensor("tp_wT", [C, 128], BF16, kind="ExternalInput")
    tp_b = nc.dram_tensor("tp_b", [128, 1], FP32, kind="ExternalInput")
    g_wT = nc.dram_tensor("g_wT", [C, C8], BF16, kind="ExternalInput")
    g_b_bc = nc.dram_tensor("g_b_bc", [128, C8], FP32, kind="ExternalInput")
    W_wTh = nc.dram_tensor("W_wTh", [C8, OC], BF16, kind="ExternalInput")
    W_bh = nc.dram_tensor("W_bh", [128, 2], FP32, kind="ExternalInput")
    ident = nc.dram_tensor("ident", [C8 + 1, C8 + 1], FP32, kind="ExternalInput")
    out = nc.dram_tensor("out", [OC, N], BF16, kind="ExternalOutput")

    y_bounce = nc.dram_tensor("y_bounce", [NH, C8], BF16)
    y_full0 = nc.dram_tensor("y_full0", [NH, C8], BF16)
    y_full1 = nc.dram_tensor("y_full1", [NH, C8], BF16)
    y_fulls = [y_full0, y_full1]

    with tile.TileContext(nc) as tc, ExitStack() as ctx:
        wpool = ctx.enter_context(tc.tile_pool(name="weights", bufs=1))
        big = ctx.enter_context(tc.tile_pool(name="big", bufs=1))

        # ---- weights to SBUF ----
        tp_w_sb = wpool.tile([128, 4, 128], BF16)
        g_w_sb = wpool.tile([128, 4, C8], BF16)
        for c in range(4):
            nc.sync.dma_start(out=tp_w_sb[:, c, :], in_=tp_wT[c * 128:(c + 1) * 128, :])
            nc.sync.dma_start(out=g_w_sb[:, c, :], in_=g_wT[c * 128:(c + 1) * 128, :])
        tp_b_sb = wpool.tile([128, 1], FP32)
        nc.sync.dma_start(out=tp_b_sb[:], in_=tp_b[:])
        g_b_sb = wpool.tile([128, C8], FP32)
        nc.sync.dma_start(out=g_b_sb[:], in_=g_b_bc[:])
        W_w_lo = wpool.tile([32, OC], BF16)
        nc.sync.dma_start(out=W_w_lo[:], in_=W_wTh[0:32, :])
        W_w_hi = wpool.tile([32, OC], BF16)
        nc.sync.dma_start(out=W_w_hi[:], in_=W_wTh[32:64, :])
        W_b_sb = wpool.tile([128, 2], FP32)
        nc.sync.dma_start(out=W_b_sb[:], in_=W_bh[:])
        id_sb = wpool.tile([C8 + 1, C8 + 1], FP32)
        nc.sync.dma_start(out=id_sb[:], in_=ident[:])

        # ---- persistent big tensors ----
        x0_sb = big.tile([128, 4, N], BF16)      # c-chunk on middle index
        x1_sb = big.tile([128, 4, N], BF16)      # c-chunk on middle index
        theta_sb = big.tile([C8, N], BF16)       # keys, [64, 4096]
        phi_sb = big.tile([C8, NH], BF16)        # queries (own half), [64, 2048]
        g_extT = big.tile([128, 32, C8 + 1], BF16)  # [m-chunk, 65] per chunk
        yT_sb = big.tile([C8 + 1, NH], FP32)
        yv_lo = big.tile([32, N], BF16)          # gather-0 rows of viewed y
        yv_hi = big.tile([32, N], BF16)          # gather-1 rows of viewed y

        nc.gpsimd.memset(g_extT[:, :, C8:C8 + 1], 1.0)

        # ---- input DMA: interleave x1/x0 blocks so both stream in early ----
        for blk in range(8):
            cols = slice(blk * 512, (blk + 1) * 512)
            for c in range(4):
                nc.sync.dma_start(out=x1_sb[:, c, cols],
                                  in_=x1[c * 128:(c + 1) * 128, cols])
            for c in range(4):
                nc.sync.dma_start(out=x0_sb[:, c, cols],
                                  in_=x0[c * 128:(c + 1) * 128, cols])

        ps_f = ctx.enter_context(tc.tile_pool(name="ps_f", bufs=2, space="PSUM"))
        ps_y = ctx.enter_context(tc.tile_pool(name="ps_y", bufs=1, space="PSUM"))
        ps_sm = ctx.enter_context(tc.tile_pool(name="ps_sm", bufs=2, space="PSUM"))
        ppool = ctx.enter_context(tc.tile_pool(name="pT", bufs=3))
        ystage = ctx.enter_context(tc.tile_pool(name="ystage", bufs=3))

        def stage1_block(blk):
            """theta/phi conv for x1 block blk (512 tokens)."""
            cols = slice(blk * 512, (blk + 1) * 512)
            ptp = ps_sm.tile([128, 512], FP32, tag="sm")
            for c in range(4):
                nc.tensor.matmul(ptp[:], tp_w_sb[:, c, :], x1_sb[:, c, cols],
                                 start=(c == 0), stop=(c == 3))
            nc.vector.tensor_scalar_add(theta_sb[:, cols], ptp[0:C8, :],
                                        tp_b_sb[0:C8, :])
            if blk < 4:
                nc.vector.tensor_scalar_add(phi_sb[:, cols], ptp[C8:128, :],
                                            tp_b_sb[C8:128, :])

        def stage2_chunk(mi):
            """g conv for token chunk mi (128 tokens), transposed layout."""
            pg = ps_sm.tile([128, 512], FP32, tag="sm")
            for c in range(4):
                nc.tensor.matmul(pg[:, 0:C8],
                                 x0_sb[:, c, mi * 128:(mi + 1) * 128],
                                 g_w_sb[:, c, :],
                                 start=(c == 0), stop=(c == 3))
            nc.vector.tensor_copy(g_extT[:, mi, 0:C8], pg[:, 0:C8])

        # blocks 0,1 of stage1 + chunks 0..3 of stage2 must precede the loop
        for blk in range(2):
            stage1_block(blk)
        for mi in range(4):
            stage2_chunk(mi)

        # ---- main attention loop, stage work interleaved into q=0 ----
        for q in range(2):
            qc = slice(q * 1024, (q + 1) * 1024)
            py = ps_y.tile([C8 + 1, 1024], FP32)
            for mi in range(32):
                if q == 0:
                    if mi % 4 == 0 and mi // 4 + 2 < 8:
                        stage1_block(mi // 4 + 2)
                    if mi + 4 < 32:
                        stage2_chunk(mi + 4)
                ft = ps_f.tile([128, 1024], FP32)
                for s in range(2):
                    nc.tensor.matmul(
                        ft[:, s * 512:(s + 1) * 512],
                        theta_sb[:, mi * 128:(mi + 1) * 128],
                        phi_sb[:, q * 1024 + s * 512: q * 1024 + (s + 1) * 512],
                        start=True, stop=True)
                pt = ppool.tile([128, 1024], BF16)
                nc.scalar.activation(pt[:], ft[:], ACTF.Exp)
                for s in range(2):
                    nc.tensor.matmul(
                        py[:, s * 512:(s + 1) * 512],
                        g_extT[:, mi, :],
                        pt[:, s * 512:(s + 1) * 512],
                        start=(mi == 0), stop=(mi == 31))

            # ---- q-half tail: transpose, normalize, exchange ----
            nc.vector.tensor_copy(yT_sb[:, qc], py[:])
            for j in range(8):
                col = q * 1024 + j * 128
                ptile = ps_sm.tile([128, 512], FP32, tag="sm")
                nc.tensor.transpose(ptile[:, 0:C8 + 1],
                                    yT_sb[:, col:col + 128], id_sb[:])
                rec = ystage.tile([128, 1], FP32, tag="rec")
                nc.vector.reciprocal(rec[:], ptile[:, C8:C8 + 1])
                yst = ystage.tile([128, C8], BF16, tag="yst")
                nc.vector.scalar_tensor_tensor(yst[:], ptile[:, 0:C8], rec[:],
                                               g_b_sb[:], op0=ALU.mult,
                                               op1=ALU.add)
                nc.sync.dma_start(out=y_bounce[col:col + 128, :], in_=yst[:])

            nc.gpsimd.collective_compute(
                "AllGather", ALU.bypass,
                replica_groups=REPLICA_PAIRS,
                ins=[y_bounce[q * 1024:(q + 1) * 1024, :]],
                outs=[y_fulls[q][:]],
            )
            yv_dst = yv_lo if q == 0 else yv_hi
            nc.sync.dma_start(
                out=yv_dst[:],
                in_=y_fulls[q][:].rearrange("(a b) w -> a (b w)", a=32))

            # x0 instance stats on DVE while q=1 attention runs
            if q == 0:
                x_aggs = []
                for oc in range(2):
                    xst = big.tile([128, 8, 6], FP32)
                    for mb in range(8):
                        nc.vector.bn_stats(xst[:, mb, :],
                                           x0_sb[:, oc, mb * 512:(mb + 1) * 512])
                    xagg = big.tile([128, 2], FP32)
                    nc.vector.bn_aggr(xagg[:], xst[:])
                    x_aggs.append(xagg)

        # ---- phase 2: W_y stats + per-channel affine + output ----
        with tc.tile_pool(name="sc", bufs=1) as sc, \
             tc.tile_pool(name="outp", bufs=2) as outp:
            for oc in range(2):
                wst = sc.tile([128, 8, 6], FP32, tag=f"wst{oc}")
                for mb in range(8):
                    cols = slice(mb * 512, (mb + 1) * 512)
                    pw = ps_sm.tile([128, 512], FP32, tag="sm")
                    nc.tensor.matmul(pw[:], W_w_lo[:, oc * 128:(oc + 1) * 128],
                                     yv_lo[:, cols], start=True, stop=False)
                    nc.tensor.matmul(pw[:], W_w_hi[:, oc * 128:(oc + 1) * 128],
                                     yv_hi[:, cols], start=False, stop=True)
                    nc.vector.bn_stats(wst[:, mb, :], pw[:])
                wagg = sc.tile([128, 2], FP32, tag=f"wagg{oc}")
                nc.vector.bn_aggr(wagg[:], wst[:])

                # r = sqrt((var_s + eps) / (var_c + eps)); t = mu_s - r*mu_c
                vc = sc.tile([128, 1], FP32, tag=f"vc{oc}")
                nc.vector.tensor_scalar_add(vc[:], x_aggs[oc][:, 1:2], EPS)
                rc = sc.tile([128, 1], FP32, tag=f"rc{oc}")
                nc.vector.reciprocal(rc[:], vc[:])
                vs = sc.tile([128, 1], FP32, tag=f"vs{oc}")
                nc.vector.tensor_scalar_add(vs[:], wagg[:, 1:2], EPS)
                ratio = sc.tile([128, 1], FP32, tag=f"ratio{oc}")
                nc.vector.tensor_mul(ratio[:], vs[:], rc[:])
                rr = sc.tile([128, 1], FP32, tag=f"rr{oc}")
                nc.scalar.sqrt(rr[:], ratio[:])
                mus = sc.tile([128, 1], FP32, tag=f"mus{oc}")
                nc.vector.tensor_add(mus[:], wagg[:, 0:1], W_b_sb[:, oc:oc + 1])
                rmc = sc.tile([128, 1], FP32, tag=f"rmc{oc}")
                nc.vector.tensor_mul(rmc[:], rr[:], x_aggs[oc][:, 0:1])
                tt = sc.tile([128, 1], FP32, tag=f"tt{oc}")
                nc.vector.tensor_sub(tt[:], mus[:], rmc[:])

                for mb in range(4):
                    cols = slice(mb * 1024, (mb + 1) * 1024)
                    ot = outp.tile([128, 1024], BF16)
                    nc.vector.tensor_scalar(ot[:], x0_sb[:, oc, cols], rr[:], tt[:],
                                            ALU.mult, ALU.add)
                    nc.sync.dma_start(out=out[oc * 128:(oc + 1) * 128, cols], in_=ot[:])

    _split_excess_waits(nc)
    return nc


_NC_CACHE = None


def _get_nc():
    global _NC_CACHE
    if _NC_CACHE is None:
        _NC_CACHE = build_nc()
    return _NC_CACHE


def _core_inputs(x0f, x1f, tp_wT, tp_b, g_wT, g_b, W_wT, W_b, ident, core):
    b, half = core // 2, core % 2
    x0b, x1b = x0f[b], x1f[b]
    if half == 0:
        x0p = x0b
        x1p = x1b
        g_wp = g_wT
    else:
        # queries-first column permutation; own-channels-first row permutation
        x1p = np.concatenate([x1b[:, NH:], x1b[:, :NH]], axis=1)
        x0r = np.concatenate([x0b[OC:], x0b[:OC]], axis=0)
        x0p = np.concatenate([x0r[:, NH:], x0r[:, :NH]], axis=1)
        g_wp = np.concatenate([g_wT[OC:], g_wT[:OC]], axis=0)
    # W rows permuted so each AllGather's rows are a contiguous yv block
    W_p = W_wT[W_ROW_PERM][:, half * OC:(half + 1) * OC]
    return {
        "x0": np.ascontiguousarray(x0p.astype(ml_dtypes.bfloat16)),
        "x1": np.ascontiguousarray(x1p.astype(ml_dtypes.bfloat16)),
        "tp_wT": tp_wT,
        "tp_b": tp_b,
        "g_wT": np.ascontiguousarray(g_wp.astype(ml_dtypes.bfloat16)),
        "g_b_bc": np.ascontiguousarray(
            np.broadcast_to(g_b, (128, C8)).astype(np.float32)),
        "W_wTh": np.ascontiguousarray(W_p.astype(ml_dtypes.bfloat16)),
        "W_bh": np.ascontiguousarray(
            W_b[half * OC:(half + 1) * OC].reshape(2, 128).T.astype(np.float32)),
        "ident": ident,
    }


def _make_in_maps(inputs):
    x0 = np.asarray(inputs["x0"], dtype=np.float32)
    x1 = np.asarray(inputs["x1"], dtype=np.float32)
    x0f = x0.reshape(B, C, N)
    x1f = x1.reshape(B, C, N)
    tp_wT = np.ascontiguousarray(
        np.concatenate([np.asarray(inputs["theta_w"], np.float32),
                        np.asarray(inputs["phi_w"], np.float32)], axis=0).T
        .astype(ml_dtypes.bfloat16))
    tp_b = np.ascontiguousarray(
        np.concatenate([np.asarray(inputs["theta_b"], np.float32),
                        np.asarray(inputs["phi_b"], np.float32)])[:, None])
    g_wT = np.ascontiguousarray(np.asarray(inputs["g_w"], np.float32).T)
    W_wT = np.ascontiguousarray(np.asarray(inputs["W_w"], np.float32).T)
    ident = np.eye(C8 + 1, dtype=np.float32)
    g_b = np.asarray(inputs["g_b"], np.float32)
    W_b = np.asarray(inputs["W_b"], np.float32)
    return [
        _core_inputs(x0f, x1f, tp_wT, tp_b, g_wT, g_b, W_wT, W_b, ident, core)
        for core in range(8)
    ]


def kernel(x0, x1, g_w, g_b, theta_w, theta_b, phi_w, phi_b, W_w, W_b):
    in_maps = _make_in_maps(dict(
        x0=x0, x1=x1, g_w=g_w, g_b=g_b, theta_w=theta_w, theta_b=theta_b,
        phi_w=phi_w, phi_b=phi_b, W_w=W_w, W_b=W_b))
    nc = _get_nc()
    res = run_bass_kernel_spmd(nc, in_maps, core_ids=list(range(8)))

    out = np.empty((B, C, N), dtype=np.float32)
    for core in range(8):
        b, half = core // 2, core % 2
        o = np.asarray(res.results[core]["out"]).astype(np.float32)
        if half == 1:
            o = np.concatenate([o[:, NH:], o[:, :NH]], axis=1)
        out[b, half * OC:(half + 1) * OC] = o
    return out.reshape(B, C, H, W)


# revision 12
# speedup vs baseline: 2.7287x; 1.0785x over previous
"""Trainium2 Bass kernel for nn_CrossAttentionBlock (B=4, C=512, H=W=64).

Decomposition across 8 NeuronCores: core = (batch b, query-half h).
v2: all matmuls in bf16 (1 cyc/row on PE vs 4 for fp32), bf16 input DMA,
conv stages interleaved into the attention loop so the in-order PE stream
never stalls on input DMA, per-query-half AllGather (bf16) so the first
exchange hides under the second half's compute.

Each core:
  interleaved: theta/phi = conv1x1(x1) (PE, bf16), gT = conv1x1(x0) rows
  main:    fT[m, n] = theta^T phi (PE), p = exp(fT) (ACT, bf16 out),
           yT_ext = [g, 1]^T p accumulated over key chunks (PE) -> softmax
           numerator rows 0..63 and denominator row 64 in one accumulation.
  per q-half: transpose yT -> y rows, normalize, + g_b, AllGather (pairwise)
  phase 2: W_y = W [view of y] consumed only as per-channel bn stats (AdaIN
           needs only mean/var of W_y); x0 instance stats; final out =
           r * x0 + t with per-channel scalars, bf16 out.

SPMD uniformity: the key/spatial axis m and the channel axis c are dummy
(contraction/stat) indices, so each core receives inputs permuted so that
"its" queries and "its" output channels come first; the host un-permutes
the output columns. W_w rows are permuted so the two AllGather chunks land
in contiguous yv row blocks.
"""
import numpy as np
import ml_dtypes
from contextlib import ExitStack

import concourse.bass as bass
import concourse.tile as tile
from concourse import mybir
from concourse.bass_utils import run_bass_kernel_spmd

FP32 = mybir.dt.float32
BF16 = mybir.dt.bfloat16
ALU = mybir.AluOpType
ACTF = mybir.ActivationFunctionType

B, C, H, W = 4, 512, 64, 64
N = H * W          # 4096 tokens
C8 = C // 8        # 64 inner channels
NH = N // 2        # 2048 queries per core
OC = C // 2        # 256 output channels per core
EPS = 1e-5

REPLICA_PAIRS = [[0, 1], [2, 3], [4, 5], [6, 7]]

# yv row blocks delivered by the two AllGathers (see _core_inputs W_p perm):
# gather q=0 -> view rows [0:16] u [32:48]; q=1 -> [16:32] u [48:64].
W_ROW_PERM = np.concatenate([
    np.arange(0, 16), np.arange(32, 48),
    np.arange(16, 32), np.arange(48, 64),
])


def _split_excess_waits(nc, max_waits=1, drain_max=1):
    """walrus here rejects instructions carrying more than ~2 sync waits; move
    extras to preceding NoOps on the same engine (semantics preserved: waits
    run before the instruction, engine streams are sequential)."""
    for blk in nc.main_func.blocks:
        insts = blk.instructions
        k = 0
        while k < len(insts):
            inst = insts[k]
            si = inst.sync_info
            cap = drain_max if inst.opcode == "Drain" else max_waits
            if si is not None and si.on_wait and len(si.on_wait) > cap:
                waits = list(si.on_wait)
                keep = waits[-cap:]
                extra = waits[:-cap]
                pos = k
                for j in range(0, len(extra), cap):
                    nop = mybir.InstNoOp(name=f"{inst.name}-wsplit{j}", ins=[], outs=[])
                    nop.engine = inst.engine
                    nop.sync_info = mybir.SyncInfo(
                        on_wait=extra[j : j + cap], on_update=[]
                    )
                    insts.insert(pos, nop)
                    pos += 1
                    k += 1
                inst.sync_info = mybir.SyncInfo(on_wait=keep, on_update=list(si.on_update))
            k += 1


def build_nc():
    nc = bass.Bass()

    x0 = nc.dram_tensor("x0", [C, N], BF16, kind="ExternalInput")
    x1 = nc.dram_tensor("x1", [C, N], BF16, kind="ExternalInput")
    tp_wT = nc.dram_tensor("tp_wT", [C, 128], BF16, kind="ExternalInput")
    tp_b = nc.dram_tensor("tp_b", [128, 1], FP32, kind="ExternalInput")
    g_wT = nc.dram_tensor("g_wT", [C, C8], BF16, kind="ExternalInput")
    g_b_bc = nc.dram_tensor("g_b_bc", [128, C8], FP32, kind="ExternalInput")
    W_wTh = nc.dram_tensor("W_wTh", [C8, OC], BF16, kind="ExternalInput")
    W_bh = nc.dram_tensor("W_bh", [128, 2], FP32, kind="ExternalInput")
    ident = nc.dram_tensor("ident", [C8 + 1, C8 + 1], FP32, kind="ExternalInput")
    out = nc.dram_tensor("out", [OC, N], BF16, kind="ExternalOutput")

    y_bounce = nc.dram_tensor("y_bounce", [NH, C8], BF16)
    y_full0 = nc.dram_tensor("y_full0", [NH, C8], BF16)
    y_full1 = nc.dram_tensor("y_full1", [NH, C8], BF16)
    y_fulls = [y_full0, y_full1]

    with tile.TileContext(nc) as tc, ExitStack() as ctx:
        wpool = ctx.enter_context(tc.tile_pool(name="weights", bufs=1))
        big = ctx.enter_context(tc.tile_pool(name="big", bufs=1))

        # ---- weights to SBUF ----
        tp_w_sb = wpool.tile([128, 4, 128], BF16)
        g_w_sb = wpool.tile([128, 4, C8], BF16)
        for c in range(4):
            nc.sync.dma_start(out=tp_w_sb[:, c, :], in_=tp_wT[c * 128:(c + 1) * 128, :])
            nc.sync.dma_start(out=g_w_sb[:, c, :], in_=g_wT[c * 128:(c + 1) * 128, :])
        tp_b_sb = wpool.tile([128, 1], FP32)
        nc.sync.dma_start(out=tp_b_sb[:], in_=tp_b[:])
        g_b_sb = wpool.tile([128, C8], FP32)
        nc.sync.dma_start(out=g_b_sb[:], in_=g_b_bc[:])
        W_w_sb = wpool.tile([C8, OC], BF16)
        nc.sync.dma_start(out=W_w_sb[:], in_=W_wTh[:])
        W_b_sb = wpool.tile([128, 2], FP32)
        nc.sync.dma_start(out=W_b_sb[:], in_=W_bh[:])
        id_sb = wpool.tile([C8 + 1, C8 + 1], FP32)
        nc.sync.dma_start(out=id_sb[:], in_=ident[:])

        # ---- persistent big tensors ----
        x0_sb = big.tile([128, 4, N], BF16)      # c-chunk on middle index
        x1_sb = big.tile([128, 4, N], BF16)      # c-chunk on middle index
        theta_sb = big.tile([C8, N], BF16)       # keys, [64, 4096]
        phi_sb = big.tile([C8, NH], BF16)        # queries (own half), [64, 2048]
        g_extT = big.tile([128, 32, C8 + 1], BF16)  # [m-chunk, 65] per chunk
        yT_sb = big.tile([C8 + 1, NH], FP32)
        yv_sb = big.tile([C8, N], BF16)          # gathered y in view-row layout

        nc.gpsimd.memset(g_extT[:, :, C8:C8 + 1], 1.0)

        # ---- input DMA: big interleaved blocks so both stream in early ----
        x1_r = x1[:].rearrange("(c p) w -> p c w", c=4)
        x0_r = x0[:].rearrange("(c p) w -> p c w", c=4)
        for blk in range(4):
            cols = slice(blk * 1024, (blk + 1) * 1024)
            nc.sync.dma_start(out=x1_sb[:, :, cols], in_=x1_r[:, :, cols])
            nc.sync.dma_start(out=x0_sb[:, :, cols], in_=x0_r[:, :, cols])

        ps_f = ctx.enter_context(tc.tile_pool(name="ps_f", bufs=2, space="PSUM"))
        ps_y = ctx.enter_context(tc.tile_pool(name="ps_y", bufs=1, space="PSUM"))
        ps_sm = ctx.enter_context(tc.tile_pool(name="ps_sm", bufs=2, space="PSUM"))
        ppool = ctx.enter_context(tc.tile_pool(name="pT", bufs=3))
        ystage = ctx.enter_context(tc.tile_pool(name="ystage", bufs=3))

        def stage1_block(blk):
            """theta/phi conv for x1 block blk (512 tokens)."""
            cols = slice(blk * 512, (blk + 1) * 512)
            ptp = ps_sm.tile([128, 512], FP32, tag="sm", name="ptp")
            for c in range(4):
                nc.tensor.matmul(ptp[:], tp_w_sb[:, c, :], x1_sb[:, c, cols],
                                 start=(c == 0), stop=(c == 3))
            nc.vector.tensor_scalar_add(theta_sb[:, cols], ptp[0:C8, :],
                                        tp_b_sb[0:C8, :])
            if blk < 4:
                nc.vector.tensor_scalar_add(phi_sb[:, cols], ptp[C8:128, :],
                                            tp_b_sb[C8:128, :])

        def stage2_chunk(mi):
            """g conv for token chunk mi (128 tokens), transposed layout."""
            pg = ps_sm.tile([128, 512], FP32, tag="sm", name="pg")
            for c in range(4):
                nc.tensor.matmul(pg[:, 0:C8],
                                 x0_sb[:, c, mi * 128:(mi + 1) * 128],
                                 g_w_sb[:, c, :],
                                 start=(c == 0), stop=(c == 3))
            nc.vector.tensor_copy(g_extT[:, mi, 0:C8], pg[:, 0:C8])

        def emit_ft(q, mi):
            ft = ps_f.tile([128, 1024], FP32, tag="ft", name="ft")
            for s in range(2):
                nc.tensor.matmul(
                    ft[:, s * 512:(s + 1) * 512],
                    theta_sb[:, mi * 128:(mi + 1) * 128],
                    phi_sb[:, q * 1024 + s * 512: q * 1024 + (s + 1) * 512],
                    start=True, stop=True)
            return ft

        def q_tail(q, py):
            """transpose, normalize, exchange for one query half."""
            qc = slice(q * 1024, (q + 1) * 1024)
            nc.vector.tensor_copy(yT_sb[:, qc], py[:])
            ybst = ystage.tile([128, 8, C8], BF16, tag="ybst", name="ybst")
            for j in range(8):
                col = q * 1024 + j * 128
                ptile = ps_sm.tile([128, 512], FP32, tag="sm", name="ptile")
                nc.tensor.transpose(ptile[:, 0:C8 + 1],
                                    yT_sb[:, col:col + 128], id_sb[:])
                rec = ystage.tile([128, 1], FP32, tag="rec", name="rec")
                nc.vector.reciprocal(rec[:], ptile[:, C8:C8 + 1])
                nc.vector.scalar_tensor_tensor(ybst[:, j, :], ptile[:, 0:C8],
                                               rec[:], g_b_sb[:], op0=ALU.mult,
                                               op1=ALU.add)
            nc.sync.dma_start(
                out=y_bounce[q * 1024:(q + 1) * 1024, :]
                    .rearrange("(j p) w -> p j w", j=8),
                in_=ybst[:])
            nc.gpsimd.collective_compute(
                "AllGather", ALU.bypass,
                replica_groups=REPLICA_PAIRS,
                ins=[y_bounce[q * 1024:(q + 1) * 1024, :]],
                outs=[y_fulls[q][:]],
            )
            nc.sync.dma_start(
                out=yv_sb[q * 32:(q + 1) * 32, :],
                in_=y_fulls[q][:].rearrange("(a b) w -> a (b w)", a=32))

        # blocks 0,1 of stage1 + chunks 0..3 of stage2 must precede the loop
        for blk in range(2):
            stage1_block(blk)
        for mi in range(4):
            stage2_chunk(mi)

        # ---- main attention loop, software-pipelined (ft one step ahead),
        # ---- stage work interleaved into q=0
        steps = [(q, mi) for q in range(2) for mi in range(32)]
        py_tiles = {0: None, 1: None}
        ft_cur = emit_ft(0, 0)
        for idx, (q, mi) in enumerate(steps):
            if q == 0:
                if mi % 4 == 0 and mi // 4 + 2 < 8:
                    stage1_block(mi // 4 + 2)
                if mi + 4 < 32:
                    stage2_chunk(mi + 4)
            if mi == 0:
                py_tiles[q] = ps_y.tile([C8 + 1, 1024], FP32, name="py")
            py = py_tiles[q]
            # prefetch next ft so PE computes it while ACT runs exp(cur)
            ft_nxt = emit_ft(*steps[idx + 1]) if idx + 1 < len(steps) else None
            pt = ppool.tile([128, 1024], BF16, name="pt")
            nc.scalar.activation(pt[:], ft_cur[:], ACTF.Exp)
            for s in range(2):
                nc.tensor.matmul(
                    py[:, s * 512:(s + 1) * 512],
                    g_extT[:, mi, :],
                    pt[:, s * 512:(s + 1) * 512],
                    start=(mi == 0), stop=(mi == 31))
            ft_cur = ft_nxt
            if mi == 31:
                q_tail(q, py)
            if q == 0 and mi == 31:
                # x0 instance stats on DVE while q=1 attention runs
                x_aggs = []
                for oc in range(2):
                    xst = big.tile([128, 8, 6], FP32, name="xst")
                    for mb in range(8):
                        nc.vector.bn_stats(xst[:, mb, :],
                                           x0_sb[:, oc, mb * 512:(mb + 1) * 512])
                    xagg = big.tile([128, 2], FP32, name="xagg")
                    nc.vector.bn_aggr(xagg[:], xst[:])
                    x_aggs.append(xagg)

        # ---- phase 2: W_y stats + per-channel affine + output ----
        with tc.tile_pool(name="sc", bufs=1) as sc, \
             tc.tile_pool(name="outp", bufs=2) as outp:
            for oc in range(2):
                wst = sc.tile([128, 8, 6], FP32, tag=f"wst{oc}")
                for mb in range(8):
                    cols = slice(mb * 512, (mb + 1) * 512)
                    pw = ps_sm.tile([128, 512], FP32, tag="sm", name="pw")
                    nc.tensor.matmul(pw[:], W_w_sb[:, oc * 128:(oc + 1) * 128],
                                     yv_sb[:, cols], start=True, stop=True)
                    nc.vector.bn_stats(wst[:, mb, :], pw[:])
                wagg = sc.tile([128, 2], FP32, tag=f"wagg{oc}")
                nc.vector.bn_aggr(wagg[:], wst[:])

                # r = sqrt((var_s + eps) / (var_c + eps)); t = mu_s - r*mu_c
                vc = sc.tile([128, 1], FP32, tag=f"vc{oc}")
                nc.vector.tensor_scalar_add(vc[:], x_aggs[oc][:, 1:2], EPS)
                rc = sc.tile([128, 1], FP32, tag=f"rc{oc}")
                nc.vector.reciprocal(rc[:], vc[:])
                vs = sc.tile([128, 1], FP32, tag=f"vs{oc}")
                nc.vector.tensor_scalar_add(vs[:], wagg[:, 1:2], EPS)
                ratio = sc.tile([128, 1], FP32, tag=f"ratio{oc}")
                nc.vector.tensor_mul(ratio[:], vs[:], rc[:])
                rr = sc.tile([128, 1], FP32, tag=f"rr{oc}")
                nc.scalar.sqrt(rr[:], ratio[:])
                mus = sc.tile([128, 1], FP32, tag=f"mus{oc}")
                nc.vector.tensor_add(mus[:], wagg[:, 0:1], W_b_sb[:, oc:oc + 1])
                rmc = sc.tile([128, 1], FP32, tag=f"rmc{oc}")
                nc.vector.tensor_mul(rmc[:], rr[:], x_aggs[oc][:, 0:1])
                tt = sc.tile([128, 1], FP32, tag=f"tt{oc}")
                nc.vector.tensor_sub(tt[:], mus[:], rmc[:])

                for mb in range(4):
                    cols = slice(mb * 1024, (mb + 1) * 1024)
                    ot = outp.tile([128, 1024], BF16, name="ot")
                    nc.vector.tensor_scalar(ot[:], x0_sb[:, oc, cols], rr[:], tt[:],
                                            ALU.mult, ALU.add)
                    nc.sync.dma_start(out=out[oc * 128:(oc + 1) * 128, cols], in_=ot[:])

    _split_excess_waits(nc)
    return nc


_NC_CACHE = None


def _get_nc():
    global _NC_CACHE
    if _NC_CACHE is None:
        _NC_CACHE = build_nc()
    return _NC_CACHE


def _core_inputs(x0f, x1f, tp_wT, tp_b, g_wT, g_b, W_wT, W_b, ident, core):
    b, half = core // 2, core % 2
    x0b, x1b = x0f[b], x1f[b]
    if half == 0:
        x0p = x0b
        x1p = x1b
        g_wp = g_wT
    else:
        # queries-first column permutation; own-channels-first row permutation
        x1p = np.concatenate([x1b[:, NH:], x1b[:, :NH]], axis=1)
        x0r = np.concatenate([x0b[OC:], x0b[:OC]], axis=0)
        x0p = np.concatenate([x0r[:, NH:], x0r[:, :NH]], axis=1)
        g_wp = np.concatenate([g_wT[OC:], g_wT[:OC]], axis=0)
    # W rows permuted so each AllGather's rows are a contiguous yv block
    W_p = W_wT[W_ROW_PERM][:, half * OC:(half + 1) * OC]
    return {
        "x0": np.ascontiguousarray(x0p.astype(ml_dtypes.bfloat16)),
        "x1": np.ascontiguousarray(x1p.astype(ml_dtypes.bfloat16)),
        "tp_wT": tp_wT,
        "tp_b": tp_b,
        "g_wT": np.ascontiguousarray(g_wp.astype(ml_dtypes.bfloat16)),
        "g_b_bc": np.ascontiguousarray(
            np.broadcast_to(g_b, (128, C8)).astype(np.float32)),
        "W_wTh": np.ascontiguousarray(W_p.astype(ml_dtypes.bfloat16)),
        "W_bh": np.ascontiguousarray(
            W_b[half * OC:(half + 1) * OC].reshape(2, 128).T.astype(np.float32)),
        "ident": ident,
    }


def _make_in_maps(inputs):
    x0 = np.asarray(inputs["x0"], dtype=np.float32)
    x1 = np.asarray(inputs["x1"], dtype=np.float32)
    x0f = x0.reshape(B, C, N)
    x1f = x1.reshape(B, C, N)
    tp_wT = np.ascontiguousarray(
        np.concatenate([np.asarray(inputs["theta_w"], np.float32),
                        np.asarray(inputs["phi_w"], np.float32)], axis=0).T
        .astype(ml_dtypes.bfloat16))
    tp_b = np.ascontiguousarray(
        np.concatenate([np.asarray(inputs["theta_b"], np.float32),
                        np.asarray(inputs["phi_b"], np.float32)])[:, None])
    g_wT = np.ascontiguousarray(np.asarray(inputs["g_w"], np.float32).T)
    W_wT = np.ascontiguousarray(np.asarray(inputs["W_w"], np.float32).T)
    ident = np.eye(C8 + 1, dtype=np.float32)
    g_b = np.asarray(inputs["g_b"], np.float32)
    W_b = np.asarray(inputs["W_b"], np.float32)
    return [
        _core_inputs(x0f, x1f, tp_wT, tp_b, g_wT, g_b, W_wT, W_b, ident, core)
        for core in range(8)
    ]


def kernel(x0, x1, g_w, g_b, theta_w, theta_b, phi_w, phi_b, W_w, W_b):
    in_maps = _make_in_maps(dict(
        x0=x0, x1=x1, g_w=g_w, g_b=g_b, theta_w=theta_w, theta_b=theta_b,
        phi_w=phi_w, phi_b=phi_b, W_w=W_w, W_b=W_b))
    nc = _get_nc()
    res = run_bass_kernel_spmd(nc, in_maps, core_ids=list(range(8)))

    out = np.empty((B, C, N), dtype=np.float32)
    for core in range(8):
        b, half = core // 2, core % 2
        o = np.asarray(res.results[core]["out"]).astype(np.float32)
        if half == 1:
            o = np.concatenate([o[:, NH:], o[:, :NH]], axis=1)
        out[b, half * OC:(half + 1) * OC] = o
    return out.reshape(B, C, H, W)


# revision 20
# speedup vs baseline: 2.7745x; 1.0168x over previous
"""Trainium2 Bass kernel for nn_CrossAttentionBlock (B=4, C=512, H=W=64).

Decomposition across 8 NeuronCores: core = (batch b, query-half h).
v2: all matmuls in bf16 (1 cyc/row on PE vs 4 for fp32), bf16 input DMA,
conv stages interleaved into the attention loop so the in-order PE stream
never stalls on input DMA, per-query-half AllGather (bf16) so the first
exchange hides under the second half's compute.

Each core:
  interleaved: theta/phi = conv1x1(x1) (PE, bf16), gT = conv1x1(x0) rows
  main:    fT[m, n] = theta^T phi (PE), p = exp(fT) (ACT, bf16 out),
           yT_ext = [g, 1]^T p accumulated over key chunks (PE) -> softmax
           numerator rows 0..63 and denominator row 64 in one accumulation.
  per q-half: transpose yT -> y rows, normalize, + g_b, AllGather (pairwise)
  phase 2: W_y = W [view of y] consumed only as per-channel bn stats (AdaIN
           needs only mean/var of W_y); x0 instance stats; final out =
           r * x0 + t with per-channel scalars, bf16 out.

SPMD uniformity: the key/spatial axis m and the channel axis c are dummy
(contraction/stat) indices, so each core receives inputs permuted so that
"its" queries and "its" output channels come first; the host un-permutes
the output columns. W_w rows are permuted so the two AllGather chunks land
in contiguous yv row blocks.
"""
import numpy as np
import ml_dtypes
from contextlib import ExitStack

import concourse.bass as bass
import concourse.tile as tile
from concourse import mybir
from concourse.bass_utils import run_bass_kernel_spmd

FP32 = mybir.dt.float32
BF16 = mybir.dt.bfloat16
ALU = mybir.AluOpType
ACTF = mybir.ActivationFunctionType

B, C, H, W = 4, 512, 64, 64
N = H * W          # 4096 tokens
C8 = C // 8        # 64 inner channels
NH = N // 2        # 2048 queries per core
OC = C // 2        # 256 output channels per core
EPS = 1e-5

REPLICA_PAIRS = [[0, 1], [2, 3], [4, 5], [6, 7]]

# yv row blocks delivered by the two AllGathers (see _core_inputs W_p perm):
# gather q=0 -> view rows [0:16] u [32:48]; q=1 -> [16:32] u [48:64].
W_ROW_PERM = np.concatenate([
    np.arange(0, 16), np.arange(32, 48),
    np.arange(16, 32), np.arange(48, 64),
])


def _split_excess_waits(nc, max_waits=1, drain_max=1):
    """walrus here rejects instructions carrying more than ~2 sync waits; move
    extras to preceding NoOps on the same engine (semantics preserved: waits
    run before the instruction, engine streams are sequential)."""
    for blk in nc.main_func.blocks:
        insts = blk.instructions
        k = 0
        while k < len(insts):
            inst = insts[k]
            si = inst.sync_info
            cap = drain_max if inst.opcode == "Drain" else max_waits
            if si is not None and si.on_wait and len(si.on_wait) > cap:
                waits = list(si.on_wait)
                keep = waits[-cap:]
                extra = waits[:-cap]
                pos = k
                for j in range(0, len(extra), cap):
                    nop = mybir.InstNoOp(name=f"{inst.name}-wsplit{j}", ins=[], outs=[])
                    nop.engine = inst.engine
                    nop.sync_info = mybir.SyncInfo(
                        on_wait=extra[j : j + cap], on_update=[]
                    )
                    insts.insert(pos, nop)
                    pos += 1
                    k += 1
                inst.sync_info = mybir.SyncInfo(on_wait=keep, on_update=list(si.on_update))
            k += 1


def build_nc():
    nc = bass.Bass()

    x0 = nc.dram_tensor("x0", [C, N], BF16, kind="ExternalInput")
    x1 = nc.dram_tensor("x1", [C, N], BF16, kind="ExternalInput")
    tp_wT = nc.dram_tensor("tp_wT", [C, 128], BF16, kind="ExternalInput")
    tp_b = nc.dram_tensor("tp_b", [128, 1], FP32, kind="ExternalInput")
    g_wT = nc.dram_tensor("g_wT", [C, C8], BF16, kind="ExternalInput")
    g_b_bc = nc.dram_tensor("g_b_bc", [128, C8], FP32, kind="ExternalInput")
    W_wTh = nc.dram_tensor("W_wTh", [C8, OC], BF16, kind="ExternalInput")
    W_bh = nc.dram_tensor("W_bh", [128, 2], FP32, kind="ExternalInput")
    out = nc.dram_tensor("out", [OC, N], BF16, kind="ExternalOutput")

    y_bounce = nc.dram_tensor("y_bounce", [NH, C8], BF16)
    y_full0 = nc.dram_tensor("y_full0", [NH, C8], BF16)
    y_full1 = nc.dram_tensor("y_full1", [NH, C8], BF16)
    y_fulls = [y_full0, y_full1]

    with tile.TileContext(nc) as tc, ExitStack() as ctx:
        wpool = ctx.enter_context(tc.tile_pool(name="weights", bufs=1))
        big = ctx.enter_context(tc.tile_pool(name="big", bufs=1))

        # ---- persistent big tensors ----
        x0_sb = big.tile([128, 4, N], BF16)      # c-chunk on middle index
        x1_sb = big.tile([128, 4, N], BF16)      # c-chunk on middle index
        theta_sb = big.tile([C8, N], BF16)       # keys, [64, 4096]
        phi_sb = big.tile([C8, NH], BF16)        # queries (own half), [64, 2048]
        g_extT = big.tile([128, 32, C8 + 1], BF16)  # [m-chunk, 65] per chunk
        yv_sb = big.tile([C8, N], BF16)          # gathered y in view-row layout

        # ---- input DMA first (the per-dma_start issue cost on SP serializes
        # all queue pushes, so order = priority), weights right after the
        # first block pair, then the rest of the inputs ----
        x1_r = x1[:].rearrange("(c p) w -> p c w", c=4)
        x0_r = x0[:].rearrange("(c p) w -> p c w", c=4)
        tp_w_sb = wpool.tile([128, 4, 128], BF16)
        g_w_sb = wpool.tile([128, 4, C8], BF16)
        tp_b_sb = wpool.tile([128, 1], FP32)
        g_b_sb = wpool.tile([128, C8], FP32)
        W_w_sb = wpool.tile([C8, OC], BF16)
        W_b_sb = wpool.tile([128, 2], FP32)

        nc.sync.dma_start(out=x1_sb[:, :, 0:1024], in_=x1_r[:, :, 0:1024])
        nc.sync.dma_start(out=tp_w_sb[:],
                          in_=tp_wT[:].rearrange("(c p) w -> p c w", c=4))
        nc.sync.dma_start(out=g_w_sb[:],
                          in_=g_wT[:].rearrange("(c p) w -> p c w", c=4))
        nc.sync.dma_start(out=tp_b_sb[:], in_=tp_b[:])
        nc.sync.dma_start(out=x0_sb[:, :, 0:1024], in_=x0_r[:, :, 0:1024])
        nc.sync.dma_start(out=g_b_sb[:], in_=g_b_bc[:])
        nc.sync.dma_start(out=W_w_sb[:], in_=W_wTh[:])
        nc.sync.dma_start(out=W_b_sb[:], in_=W_bh[:])
        for blk in range(1, 4):
            cols = slice(blk * 1024, (blk + 1) * 1024)
            nc.sync.dma_start(out=x1_sb[:, :, cols], in_=x1_r[:, :, cols])
            nc.sync.dma_start(out=x0_sb[:, :, cols], in_=x0_r[:, :, cols])

        nc.gpsimd.memset(g_extT[:, :, C8:C8 + 1], 1.0)

        ps_f = ctx.enter_context(tc.tile_pool(name="ps_f", bufs=2, space="PSUM"))
        ps_y = ctx.enter_context(tc.tile_pool(name="ps_y", bufs=1, space="PSUM"))
        ps_sm = ctx.enter_context(tc.tile_pool(name="ps_sm", bufs=2, space="PSUM"))
        ppool = ctx.enter_context(tc.tile_pool(name="pT", bufs=3))
        ystage = ctx.enter_context(tc.tile_pool(name="ystage", bufs=3))

        def stage1_block(blk):
            """theta/phi conv for x1 block blk (512 tokens)."""
            cols = slice(blk * 512, (blk + 1) * 512)
            ptp = ps_sm.tile([128, 512], FP32, tag="sm", name="ptp")
            for c in range(4):
                nc.tensor.matmul(ptp[:], tp_w_sb[:, c, :], x1_sb[:, c, cols],
                                 start=(c == 0), stop=(c == 3))
            nc.vector.tensor_scalar_add(theta_sb[:, cols], ptp[0:C8, :],
                                        tp_b_sb[0:C8, :])
            if blk < 4:
                nc.vector.tensor_scalar_add(phi_sb[:, cols], ptp[C8:128, :],
                                            tp_b_sb[C8:128, :])

        def stage2_chunk(mi):
            """g conv for token chunk mi (128 tokens), transposed layout."""
            pg = ps_sm.tile([128, 512], FP32, tag="sm", name="pg")
            for c in range(4):
                nc.tensor.matmul(pg[:, 0:C8],
                                 x0_sb[:, c, mi * 128:(mi + 1) * 128],
                                 g_w_sb[:, c, :],
                                 start=(c == 0), stop=(c == 3))
            nc.vector.tensor_copy(g_extT[:, mi, 0:C8], pg[:, 0:C8])

        def emit_ft(q, mi):
            ft = ps_f.tile([128, 1024], FP32, tag="ft", name="ft")
            for s in range(2):
                nc.tensor.matmul(
                    ft[:, s * 512:(s + 1) * 512],
                    theta_sb[:, mi * 128:(mi + 1) * 128],
                    phi_sb[:, q * 1024 + s * 512: q * 1024 + (s + 1) * 512],
                    start=True, stop=True)
            return ft

        def q_tail(q, py):
            """normalize (py rows are already token-major), exchange."""
            ybst = ystage.tile([128, 8, C8], BF16, tag="ybst", name="ybst")
            for j in range(8):
                rec = ystage.tile([128, 1], FP32, tag="rec", name="rec")
                nc.vector.reciprocal(rec[:], py[:, j, C8:C8 + 1])
                nc.vector.scalar_tensor_tensor(ybst[:, j, :], py[:, j, 0:C8],
                                               rec[:], g_b_sb[:], op0=ALU.mult,
                                               op1=ALU.add)
            nc.sync.dma_start(
                out=y_bounce[q * 1024:(q + 1) * 1024, :]
                    .rearrange("(j p) w -> p j w", j=8),
                in_=ybst[:])
            nc.gpsimd.collective_compute(
                "AllGather", ALU.bypass,
                replica_groups=REPLICA_PAIRS,
                ins=[y_bounce[q * 1024:(q + 1) * 1024, :]],
                outs=[y_fulls[q][:]],
            )
            nc.sync.dma_start(
                out=yv_sb[q * 32:(q + 1) * 32, :],
                in_=y_fulls[q][:].rearrange("(a b) w -> a (b w)", a=32))

        # blocks 0,1 of stage1 + chunks 0..3 of stage2 must precede the loop
        for blk in range(2):
            stage1_block(blk)
        for mi in range(4):
            stage2_chunk(mi)

        # ---- main attention loop, software-pipelined (ft one step ahead),
        # ---- stage work interleaved into q=0
        steps = [(q, mi) for q in range(2) for mi in range(32)]
        py_tiles = {0: None, 1: None}
        ft_cur = emit_ft(0, 0)
        for idx, (q, mi) in enumerate(steps):
            if q == 0:
                if mi % 4 == 0 and mi // 4 + 2 < 8:
                    stage1_block(mi // 4 + 2)
                if mi + 4 < 32:
                    stage2_chunk(mi + 4)
            if mi == 0:
                # token-major y accumulator: 8 query chunks of [128, 65],
                # padded to 128-col stride so no chunk crosses a PSUM bank
                py_tiles[q] = ps_y.tile([128, 8, 128], FP32, name="py")
            py = py_tiles[q]
            # prefetch next ft so PE computes it while ACT runs exp(cur)
            ft_nxt = emit_ft(*steps[idx + 1]) if idx + 1 < len(steps) else None
            pt = ppool.tile([128, 1024], BF16, name="pt")
            nc.scalar.activation(pt[:], ft_cur[:], ACTF.Exp)
            # the 8 chunks share two PSUM banks (zero regions): start zeroes
            # a whole 2KB bank, so only the first chunk in each bank starts
            # the group and only the last one stops it
            for j in range(8):
                nc.tensor.matmul(
                    py[:, j, 0:C8 + 1],
                    pt[:, j * 128:(j + 1) * 128],
                    g_extT[:, mi, :],
                    start=(mi == 0 and j % 4 == 0),
                    stop=(mi == 31 and j % 4 == 3))
            ft_cur = ft_nxt
            if mi == 31:
                q_tail(q, py)
            if q == 0 and mi == 31:
                # x0 instance stats + content-side scalars on DVE while
                # q=1 attention runs
                x_agg = big.tile([128, 2, 2], FP32, name="x_agg")
                for oc in range(2):
                    xst = big.tile([128, 8, 6], FP32, name="xst")
                    for mb in range(8):
                        nc.vector.bn_stats(xst[:, mb, :],
                                           x0_sb[:, oc, mb * 512:(mb + 1) * 512])
                    nc.vector.bn_aggr(x_agg[:, oc, :], xst[:])
                vc_b = big.tile([128, 2], FP32, name="vc_b")
                nc.vector.tensor_scalar_add(vc_b[:], x_agg[:, :, 1], EPS)
                rc_b = big.tile([128, 2], FP32, name="rc_b")
                nc.vector.reciprocal(rc_b[:], vc_b[:])

        # ---- phase 2: W_y stats + per-channel affine + output ----
        with tc.tile_pool(name="sc", bufs=1) as sc, \
             tc.tile_pool(name="outp", bufs=4) as outp:
            # style stats: pw tiles pipelined PE -> DVE bn_stats
            w_agg = sc.tile([128, 2, 2], FP32, name="w_agg")
            for oc in range(2):
                wst = sc.tile([128, 8, 6], FP32, tag=f"wst{oc}", name="wst")
                for mb in range(8):
                    cols = slice(mb * 512, (mb + 1) * 512)
                    pw = ps_sm.tile([128, 512], FP32, tag="sm", name="pw")
                    nc.tensor.matmul(pw[:], W_w_sb[:, oc * 128:(oc + 1) * 128],
                                     yv_sb[:, cols], start=True, stop=True)
                    nc.vector.bn_stats(wst[:, mb, :], pw[:])
                nc.vector.bn_aggr(w_agg[:, oc, :], wst[:])

            # r = sqrt((var_s + eps) / (var_c + eps)); t = mu_s - r*mu_c
            # (content-side 1/(var_c+eps) was precomputed during q=1)
            vs_b = sc.tile([128, 2], FP32, name="vs_b")
            nc.vector.tensor_scalar_add(vs_b[:], w_agg[:, :, 1], EPS)
            ratio_b = sc.tile([128, 2], FP32, name="ratio_b")
            nc.vector.tensor_mul(ratio_b[:], vs_b[:], rc_b[:])
            rr_b = sc.tile([128, 2], FP32, name="rr_b")
            nc.scalar.sqrt(rr_b[:], ratio_b[:])
            mus_b = sc.tile([128, 2], FP32, name="mus_b")
            nc.vector.tensor_add(mus_b[:], w_agg[:, :, 0], W_b_sb[:])
            rmc_b = sc.tile([128, 2], FP32, name="rmc_b")
            nc.vector.tensor_mul(rmc_b[:], rr_b[:], x_agg[:, :, 0])
            tt_b = sc.tile([128, 2], FP32, name="tt_b")
            nc.vector.tensor_sub(tt_b[:], mus_b[:], rmc_b[:])

            # final affine split across ACT/DVE/GPSIMD so it drains in
            # parallel with the output DMA
            for k in range(8):
                oc, mb = k // 4, k % 4
                cols = slice(mb * 1024, (mb + 1) * 1024)
                ot = outp.tile([128, 1024], BF16, name="ot")
                rr_ap = rr_b[:, oc:oc + 1]
                tt_ap = tt_b[:, oc:oc + 1]
                eng = (nc.scalar, nc.vector, nc.scalar, nc.gpsimd,
                       nc.vector, nc.scalar, nc.vector, nc.gpsimd)[k]
                if eng is nc.scalar:
                    nc.scalar.activation(ot[:], x0_sb[:, oc, cols],
                                         ACTF.Identity, bias=tt_ap, scale=rr_ap)
                else:
                    eng.tensor_scalar(ot[:], x0_sb[:, oc, cols], rr_ap, tt_ap,
                                      ALU.mult, ALU.add)
                nc.sync.dma_start(out=out[oc * 128:(oc + 1) * 128, cols], in_=ot[:])

    _split_excess_waits(nc)
    return nc


_NC_CACHE = None


def _get_nc():
    global _NC_CACHE
    if _NC_CACHE is None:
        _NC_CACHE = build_nc()
    return _NC_CACHE


def _core_inputs(x0f, x1f, tp_wT, tp_b, g_wT, g_b, W_wT, W_b, core):
    b, half = core // 2, core % 2
    x0b, x1b = x0f[b], x1f[b]
    if half == 0:
        x0p = x0b
        x1p = x1b
        g_wp = g_wT
    else:
        # queries-first column permutation; own-channels-first row permutation
        x1p = np.concatenate([x1b[:, NH:], x1b[:, :NH]], axis=1)
        x0r = np.concatenate([x0b[OC:], x0b[:OC]], axis=0)
        x0p = np.concatenate([x0r[:, NH:], x0r[:, :NH]], axis=1)
        g_wp = np.concatenate([g_wT[OC:], g_wT[:OC]], axis=0)
    # W rows permuted so each AllGather's rows are a contiguous yv block
    W_p = W_wT[W_ROW_PERM][:, half * OC:(half + 1) * OC]
    return {
        "x0": np.ascontiguousarray(x0p.astype(ml_dtypes.bfloat16)),
        "x1": np.ascontiguousarray(x1p.astype(ml_dtypes.bfloat16)),
        "tp_wT": tp_wT,
        "tp_b": tp_b,
        "g_wT": np.ascontiguousarray(g_wp.astype(ml_dtypes.bfloat16)),
        "g_b_bc": np.ascontiguousarray(
            np.broadcast_to(g_b, (128, C8)).astype(np.float32)),
        "W_wTh": np.ascontiguousarray(W_p.astype(ml_dtypes.bfloat16)),
        "W_bh": np.ascontiguousarray(
            W_b[half * OC:(half + 1) * OC].reshape(2, 128).T.astype(np.float32)),
    }


def _make_in_maps(inputs):
    x0 = np.asarray(inputs["x0"], dtype=np.float32)
    x1 = np.asarray(inputs["x1"], dtype=np.float32)
    x0f = x0.reshape(B, C, N)
    x1f = x1.reshape(B, C, N)
    tp_wT = np.ascontiguousarray(
        np.concatenate([np.asarray(inputs["theta_w"], np.float32),
                        np.asarray(inputs["phi_w"], np.float32)], axis=0).T
        .astype(ml_dtypes.bfloat16))
    tp_b = np.ascontiguousarray(
        np.concatenate([np.asarray(inputs["theta_b"], np.float32),
                        np.asarray(inputs["phi_b"], np.float32)])[:, None])
    g_wT = np.ascontiguousarray(np.asarray(inputs["g_w"], np.float32).T)
    W_wT = np.ascontiguousarray(np.asarray(inputs["W_w"], np.float32).T)
    g_b = np.asarray(inputs["g_b"], np.float32)
    W_b = np.asarray(inputs["W_b"], np.float32)
    return [
        _core_inputs(x0f, x1f, tp_wT, tp_b, g_wT, g_b, W_wT, W_b, core)
        for core in range(8)
    ]


def kernel(x0, x1, g_w, g_b, theta_w, theta_b, phi_w, phi_b, W_w, W_b):
    in_maps = _make_in_maps(dict(
        x0=x0, x1=x1, g_w=g_w, g_b=g_b, theta_w=theta_w, theta_b=theta_b,
        phi_w=phi_w, phi_b=phi_b, W_w=W_w, W_b=W_b))
    nc = _get_nc()
    res = run_bass_kernel_spmd(nc, in_maps, core_ids=list(range(8)))

    out = np.empty((B, C, N), dtype=np.float32)
    for core in range(8):
        b, half = core // 2, core % 2
        o = np.asarray(res.results[core]["out"]).astype(np.float32)
        if half == 1:
            o = np.concatenate([o[:, NH:], o[:, :NH]], axis=1)
        out[b, half * OC:(half + 1) * OC] = o
    return out.reshape(B, C, H, W)
